# revision 20
# baseline (speedup 1.0000x reference)
"""Trainium2 Bass kernel for CausalGatedD2Attention.

Math (per batch b):
  xn   = LayerNorm(x) * ln_g + ln_b            [T, D]
  qkv  = xn @ qkv_w + qkv_b                     -> q, k, v  [T, D] each
  gate = sigmoid(xn @ gate_w + gate_b)
  k    = elu(k * gate) + 1 ;  q = elu(q) + 1
  attn = tril(q @ k^T)                          [T, T]
  out  = (attn @ v) / (rowsum(attn) + eps)      [T, D]
(rowsum(attn) == sum(q * cumsum(k), -1) under the causal mask.)

Sharding: 4 batches x 2 cores.  Within a pair, core parity par in {0,1}
owns the even/odd 128-row t-chunks of its batch (balances the causal
triangle).  Each core LayerNorms + projects ONLY its own 1024 rows;
k (gated, elu+1) and v (with an appended [1,0] denominator column) for
the peer's rows arrive via a pair-wise DRAM AllGather.  All 8 cores run
ONE uniform program; causality is applied via two host-provided
[128,128] mask tiles (content depends only on core parity), so the
instruction stream is identical across cores - only input data differs.

Weights are folded with ln_g/ln_b on the host, cast to fp16, and
shipped as a per-core 1 MB column shard of the SBUF-layout weight
buffer; an 8-core AllGather reassembles the full 8 MB on every device
at kernel start (overlapped with the LayerNorm phase).  Biases are tiny
and embedded in the NEFF as Const tensors.  All matmul operands live in
fp16 (1 cycle/row on the PE, same as bf16, with 8 more mantissa bits);
LayerNorm statistics, activations and the num/den accumulators stay
f32.  Per-call wire traffic is xq (the core's own 1024 rows, f32,
passed as a strided view of x), the 1 MB weight shard, a 128 KB mask
pair in, and the 2 MB fp16 output slice back -- ~57 MB total vs
~296 MB for the naive replication (the host-side assembly casts fp16 ->
f32 during the strided scatter, so the down-cast is free).  The jax
persistent compilation cache is enabled so warm calls skip the backend
compile (bir verify + neuronx-cc) entirely.

The denominator comes for free: v gets an appended ones-column, so
attn @ v_aug yields [num | den] in one accumulation.
"""

import sys

sys.path.insert(0, "/opt/trn_rl_repo")

import numpy as np

B, T, D = 4, 2048, 1024
P = 128
KD = D // P          # 8 contraction chunks
NT = T // P          # 16 global t-chunks
NL = NT // 2         # 8 local t-chunks per core
LN_EPS = 1e-5
DEN_EPS = 1e-6
N_CORES = 8

_CACHE = {}


def _patched_tc(tile_mod):
    import bass_rust as _br
    from concourse.vector_clock import ScopedClock

    class TC(tile_mod.TileContext):
        """TileContext whose final drain splits sem waits one per
        instruction (walrus CoreV3 allows a single wait on Drain)."""

        def _spread_waits(self):
            # walrus allows at most 2 sem waits on engine instructions and
            # only 1 on CTRL-class ones (Drain/NoOp); Tile's scheduler can
            # emit more.  Move excess waits onto same-engine nops placed
            # immediately before the over-limit instruction.
            nc = self.nc
            for fnbb in nc.m.functions[0].blocks:
                insts = list(fnbb.instructions)
                out = []
                for inst in insts:
                    si = inst.sync_info
                    waits = list(si.on_wait) if si is not None else []
                    limit = 1
                    if len(waits) > limit:
                        excess = waits[limit:]
                        si.on_wait = waits[:limit]
                        inst.sync_info = si
                        for w in excess:
                            nop = nc.engines[inst.engine].nop(
                                nofuse=True, hint="wait_spread"
                            )
                            nop.ins.sync_info = _br.SyncInfo(
                                on_wait=[w], on_update=[]
                            )
                            # remove from wherever it was appended
                            for b2 in nc.m.functions[0].blocks:
                                cur = list(b2.instructions)
                                if cur and cur[-1] is nop.ins:
                                    b2.instructions = cur[:-1]
                                    break
                            out.append(nop.ins)
                    out.append(inst)
                fnbb.instructions = out

        def _drain_and_barrier(self, tick_clock, wait_clock):
            self._spread_waits()
            drain_inst = self.nc.sync.drain()
            wait_clock.add_sem_waits(
                drain_inst.ins, ScopedClock({None: tick_clock.global_clock})
            )
            si = drain_inst.ins.sync_info
            waits = list(si.on_wait)
            if len(waits) > 1:
                si.on_wait = waits[:1]
                drain_inst.ins.sync_info = si
                for i in range(1, len(waits)):
                    nop = self.nc.sync.nop(nofuse=True, hint="drain_extra_waits")
                    nop.ins.sync_info = _br.SyncInfo(
                        on_wait=waits[i : i + 1], on_update=[]
                    )
            self.nc.all_engine_barrier()
            assert self.sems is not None
            popped = self.nc._tile_sem_poison_stack.pop()
            assert popped is self._sem_poison
            self.nc.clear_and_free_semaphores(list(self.sems.allocated().values()))
            self.nc.all_engine_barrier()

    return TC


def build_program(wq, wk, wg, wv, bq, bk, bg, bv, use_cc=True):
    """wq/wk/wg: [KD,KD,P,P] f32 tiles (m,k);  wv: [KD,P,D] f32;
    bq/bk/bg: [D] f32;  bv: [D] f32."""
    import concourse.bass as bass
    import concourse.tile as tile
    from concourse import mybir
    from concourse.masks import make_identity

    TC = _patched_tc(tile)
    f32 = mybir.dt.float32
    f16 = mybir.dt.float16
    Act = mybir.ActivationFunctionType
    Alu = mybir.AluOpType
    bfn = np.float16

    nc = bass.Bass(num_devices=N_CORES)
    xq_in = nc.declare_dram_parameter("xq", [NL, P, D], f32, isOutput=False)
    masks_in = nc.declare_dram_parameter("masks", [2, P, P], f32, isOutput=False)
    out_d = nc.declare_dram_parameter("out", [NL * P, D], f16, isOutput=True)
    if not use_cc:
        x_in = nc.declare_dram_parameter("x", [NT, P, D], f32, isOutput=False)

    # weights pre-arranged for single-DMA SBUF residency:
    #   [m,k,p,q] -> [p, (m k q)]   (lhsT tile (m,k) = [:, (m*KD+k)*P : +P])
    def sb_order(wt):
        return np.ascontiguousarray(
            wt.transpose(2, 0, 1, 3).reshape(P, KD * KD * P).astype(bfn)
        )

    WSH = 4 * KD * KD * P // N_CORES  # 4096-col shard of the weight buffer
    if use_cc:
        # weights arrive as a per-core 1 MB fp16 shard and are AllGathered
        # on device; [wq | wk | wg | wv] each spans two 4096-col shards.
        wsh_in = nc.declare_dram_parameter("wsh", [P, WSH], f16, isOutput=False)
        own_w = nc.dram_tensor("own_w", [P, WSH], f16)
        g_w = nc.dram_tensor("g_w", [N_CORES, P, WSH], f16, addr_space="Shared")
    else:
        wq_i = nc.inline_tensor(sb_order(wq), name="wq_i")
        wk_i = nc.inline_tensor(sb_order(wk), name="wk_i")
        wg_i = nc.inline_tensor(sb_order(wg), name="wg_i")
        wv_i = nc.inline_tensor(
            np.ascontiguousarray(
                wv.transpose(1, 0, 2).reshape(P, KD * D).astype(bfn)
            ),
            name="wv_i",
        )
    # biases: [P, KD] with column m = bias[m*128:(m+1)*128]
    bq_i = nc.inline_tensor(np.ascontiguousarray(bq.reshape(KD, P).T), name="bq_i")
    bk_i = nc.inline_tensor(np.ascontiguousarray(bk.reshape(KD, P).T), name="bk_i")
    bg_i = nc.inline_tensor(np.ascontiguousarray(bg.reshape(KD, P).T), name="bg_i")
    bv_i = nc.inline_tensor(np.ascontiguousarray(bv.reshape(1, D)), name="bv_i")

    own_kT = nc.dram_tensor("own_kT", [KD, P, NL * P], f16)
    own_v = nc.dram_tensor("own_v", [NL, P, D + 2], f16)
    if use_cc:
        g_kT = nc.dram_tensor("g_kT", [2, KD, P, NL * P], f16)
        g_v = nc.dram_tensor("g_v", [2, NL, P, D + 2], f16)
        groups = [[2 * i, 2 * i + 1] for i in range(4)]
        groups8 = [list(range(N_CORES))]
    else:
        vdram = nc.dram_tensor("vdram", [NT, P, D + 2], f16)

    with TC(nc) as tc:
        if use_cc:
            # kick off the weight AllGather first; it overlaps with LNQ
            nc.sync.dma_start(out=own_w[:], in_=wsh_in[:])
            nc.gpsimd.collective_compute(
                "AllGather",
                mybir.AluOpType.bypass,
                replica_groups=groups8,
                ins=[own_w[:].opt()],
                outs=[g_w[:].opt()],
            )

        def load_w_sb(dst, widx):
            """dst [P, 8192] <- weight widx (0=q,1=k,2=g,3=v)."""
            if use_cc:
                for h in range(2):
                    nc.sync.dma_start(
                        out=dst[:, h * WSH : (h + 1) * WSH],
                        in_=g_w[2 * widx + h],
                    )
            else:
                nc.sync.dma_start(
                    out=dst, in_=(wq_i, wk_i, wg_i, wv_i)[widx][:]
                )

        const = tc.alloc_tile_pool(name="const", bufs=1)
        ident = const.tile([P, P], f16, tag="ident")
        make_identity(nc, ident)
        mask_sb = const.tile([P, 2 * P], f32, tag="mask")
        for rel in range(2):
            nc.sync.dma_start(
                out=mask_sb[:, rel * P : (rel + 1) * P], in_=masks_in[rel]
            )
        bq_sb = const.tile([P, KD], f32, tag="bq")
        bk_sb = const.tile([P, KD], f32, tag="bk")
        bg_sb = const.tile([P, KD], f32, tag="bgs")
        nc.sync.dma_start(out=bq_sb, in_=bq_i[:])
        nc.sync.dma_start(out=bk_sb, in_=bk_i[:])
        nc.sync.dma_start(out=bg_sb, in_=bg_i[:])
        vb_sb = const.tile([P, D], f32, tag="vb")
        bvs = bv_i[0]
        vb_bcast = bass.AP(tensor=bvs.tensor, offset=bvs.offset, ap=[[0, P], *bvs.ap])
        nc.sync.dma_start(out=vb_sb, in_=vb_bcast)
        ln_eps = const.tile([P, 1], f32, tag="lneps")
        nc.vector.memset(ln_eps, LN_EPS)
        onez_sb = const.tile([P, 2], f16, tag="onez")
        nc.vector.memset(onez_sb[:, 0:1], 1.0)
        nc.vector.memset(onez_sb[:, 1:2], 0.0)

        # ---- helper: layernorm one 128-row chunk + transpose into dstT ----
        def ln_transpose(src, c, dstT, xpool, spool, pspool):
            xt = xpool.tile([P, D], f32, tag="xt")
            nc.sync.dma_start(out=xt, in_=src)
            stats = spool.tile([P, 2, 6], f32, tag="stats")
            xr = xt.rearrange("p (n f) -> p n f", n=2)
            for sg in range(2):
                nc.vector.bn_stats(out=stats[:, sg], in_=xr[:, sg])
            mv = spool.tile([P, 2], f32, tag="mv")
            nc.vector.bn_aggr(out=mv, in_=stats)
            rstd = spool.tile([P, 1], f32, tag="rstd")
            nc.scalar.activation(
                out=rstd, in_=mv[:, 1:2], func=Act.Sqrt, bias=ln_eps, scale=1.0
            )
            rstd2 = spool.tile([P, 1], f32, tag="rstd2")
            nc.vector.reciprocal(out=rstd2, in_=rstd)
            nmr = spool.tile([P, 1], f32, tag="nmr")
            nc.vector.tensor_scalar(
                out=nmr,
                in0=mv[:, 0:1],
                scalar1=rstd2,
                scalar2=-1.0,
                op0=Alu.mult,
                op1=Alu.mult,
            )
            xn = xpool.tile([P, D], f16, tag="xn")
            nc.scalar.activation(
                out=xn, in_=xt, func=Act.Identity, bias=nmr, scale=rstd2
            )
            for k in range(KD):
                ps = pspool.tile([P, P], f16, tag="psT")
                nc.tensor.transpose(
                    out=ps, in_=xn[:, k * P : (k + 1) * P], identity=ident
                )
                if k % 2 == 0:
                    nc.vector.tensor_copy(dstT[k][:, c * P : (c + 1) * P], ps)
                else:
                    nc.scalar.copy(out=dstT[k][:, c * P : (c + 1) * P], in_=ps)

        # ---- helper: x_elu = elu(x)+1 = max(x,0) + exp(min(x,0)) ----
        def elu1(dst, src, epool):
            m0 = epool.tile(list(src.shape), f32, tag="m0")
            nc.gpsimd.tensor_scalar_min(out=m0, in0=src, scalar1=0.0)
            e = epool.tile(list(src.shape), f32, tag="e")
            nc.scalar.activation(out=e, in_=m0, func=Act.Exp)
            nc.vector.scalar_tensor_tensor(
                out=dst, in0=src, scalar=0.0, in1=e, op0=Alu.max, op1=Alu.add
            )

        # =========== phase LNQ: layernorm + transpose xq -> xqnT =========
        xqnT_pool = tc.alloc_tile_pool(name="xqnT", bufs=1)
        xqnT = [
            xqnT_pool.tile([P, NL * P], f16, tag=f"xqnT{k}", name=f"xqnT{k}")
            for k in range(KD)
        ]
        if not use_cc:
            xnT_pool = tc.alloc_tile_pool(name="xnT", bufs=1)
            xnT = [
                xnT_pool.tile([P, T], f16, tag=f"xnT{k}", name=f"xnT{k}")
                for k in range(KD)
            ]
        xpool = tc.alloc_tile_pool(name="qwork", bufs=3)
        spool = tc.alloc_tile_pool(name="qstat", bufs=4)
        pspool = tc.alloc_tile_pool(name="psTq", bufs=4, space="PSUM")
        for c in range(NL):
            ln_transpose(xq_in[c], c, xqnT, xpool, spool, pspool)
        if not use_cc:
            for c in range(NT):
                ln_transpose(x_in[c], c, xnT, xpool, spool, pspool)
        pspool.release()
        spool.release()
        xpool.release()

        kg_src = xqnT if use_cc else xnT
        kg_cols = NL * P if use_cc else T
        kg_sc = kg_cols // 512

        # kT holds the FULL 2048 key columns in global chunk order
        kT_pool = tc.alloc_tile_pool(name="kT", bufs=1, side="right")
        kT = [
            kT_pool.tile([P, T], f16, tag=f"kT{m}", name=f"kT{m}")
            for m in range(KD)
        ]

        # =========== phase KG: k/gate projections (own rows) =============
        wkg_pool = tc.alloc_tile_pool(name="wkg", bufs=1)
        wk_sb = wkg_pool.tile([P, KD * KD * P], f16, tag="wk_sb", name="wk_sb")
        wg_sb = wkg_pool.tile([P, KD * KD * P], f16, tag="wg_sb", name="wg_sb")
        load_w_sb(wk_sb, 1)
        load_w_sb(wg_sb, 2)
        epool = tc.alloc_tile_pool(name="kgev", bufs=2)
        kopool = tc.alloc_tile_pool(name="kout", bufs=2)
        pskg = tc.alloc_tile_pool(name="psKG", bufs=1, space="PSUM")
        for m in range(KD):
            psK = pskg.tile([P, kg_sc, 512], f32, tag="psK")
            psG = pskg.tile([P, kg_sc, 512], f32, tag="psG")
            for k in range(KD):
                wkt = wk_sb[:, (m * KD + k) * P : (m * KD + k + 1) * P]
                wgt = wg_sb[:, (m * KD + k) * P : (m * KD + k + 1) * P]
                for sc in range(kg_sc):
                    nc.tensor.matmul(
                        out=psK[:, sc],
                        lhsT=wkt,
                        rhs=kg_src[k][:, sc * 512 : (sc + 1) * 512],
                        start=(k == 0),
                        stop=(k == KD - 1),
                    )
                    nc.tensor.matmul(
                        out=psG[:, sc],
                        lhsT=wgt,
                        rhs=kg_src[k][:, sc * 512 : (sc + 1) * 512],
                        start=(k == 0),
                        stop=(k == KD - 1),
                    )
            if use_cc:
                kto = kopool.tile([P, kg_cols], f16, tag="kto", name="kto")
            else:
                kto = kT[m]
            psKf = psK.rearrange("p s c -> p (s c)")
            psGf = psG.rearrange("p s c -> p (s c)")
            for sc in range(kg_sc // 2):
                cols = slice(sc * 1024, (sc + 1) * 1024)
                g = epool.tile([P, 1024], f32, tag="g")
                nc.scalar.activation(
                    out=g,
                    in_=psGf[:, cols],
                    func=Act.Sigmoid,
                    bias=bg_sb[:, m : m + 1],
                    scale=1.0,
                )
                kg = epool.tile([P, 1024], f32, tag="kg")
                nc.vector.scalar_tensor_tensor(
                    out=kg,
                    in0=psKf[:, cols],
                    scalar=bk_sb[:, m : m + 1],
                    in1=g,
                    op0=Alu.add,
                    op1=Alu.mult,
                )
                elu1(kto[:, cols], kg, epool)
            if use_cc:
                nc.sync.dma_start(out=own_kT[m], in_=kto)
        pskg.release()
        kopool.release()
        epool.release()
        wkg_pool.release()
        if use_cc:
            nc.gpsimd.collective_compute(
                "AllGather",
                mybir.AluOpType.bypass,
                replica_groups=groups,
                ins=[own_kT[:].opt()],
                outs=[g_kT[:].opt()],
            )

        # =========== phase V: v projection (own rows) ====================
        wv_pool = tc.alloc_tile_pool(name="wv", bufs=1)
        wv_sb = wv_pool.tile([P, KD * D], f16, tag="wv_sb", name="wv_sb")
        load_w_sb(wv_sb, 3)
        vpool = tc.alloc_tile_pool(name="vev", bufs=3)
        psv = tc.alloc_tile_pool(name="psV", bufs=3, space="PSUM")
        v_src = xqnT if use_cc else xnT
        v_chunks = NL if use_cc else NT
        for s in range(v_chunks):
            ps = psv.tile([P, D], f32, tag="psV")
            for k in range(KD):
                for dc in range(2):
                    nc.tensor.matmul(
                        out=ps[:, dc * 512 : (dc + 1) * 512],
                        lhsT=v_src[k][:, s * P : (s + 1) * P],
                        rhs=wv_sb[:, k * D + dc * 512 : k * D + (dc + 1) * 512],
                        start=(k == 0),
                        stop=(k == KD - 1),
                    )
            vsb = vpool.tile([P, D + 2], f16, tag="vsb")
            nc.vector.tensor_add(vsb[:, 0:D], ps, vb_sb)
            nc.vector.tensor_copy(vsb[:, D : D + 2], onez_sb)
            nc.sync.dma_start(out=own_v[s] if use_cc else vdram[s], in_=vsb)
        psv.release()
        vpool.release()
        wv_pool.release()
        if use_cc:
            nc.gpsimd.collective_compute(
                "AllGather",
                mybir.AluOpType.bypass,
                replica_groups=groups,
                ins=[own_v[:].opt()],
                outs=[g_v[:].opt()],
            )

        # =========== phase QP: q projection -> qT (elu+1) ================
        qT_pool = tc.alloc_tile_pool(name="qT", bufs=1, side="right")
        qT = [
            qT_pool.tile([P, NL * P], f16, tag=f"qT{m}", name=f"qT{m}")
            for m in range(KD)
        ]
        wq_pool = tc.alloc_tile_pool(name="wq", bufs=1)
        wq_sb = wq_pool.tile([P, KD * KD * P], f16, tag="wq_sb", name="wq_sb")
        load_w_sb(wq_sb, 0)
        epool = tc.alloc_tile_pool(name="qev", bufs=3)
        psq = tc.alloc_tile_pool(name="psQ", bufs=3, space="PSUM")
        for m in range(KD):
            ps = psq.tile([P, NL * P], f32, tag="psQ")
            for k in range(KD):
                wqt = wq_sb[:, (m * KD + k) * P : (m * KD + k + 1) * P]
                for sc in range(2):
                    nc.tensor.matmul(
                        out=ps[:, sc * 512 : (sc + 1) * 512],
                        lhsT=wqt,
                        rhs=xqnT[k][:, sc * 512 : (sc + 1) * 512],
                        start=(k == 0),
                        stop=(k == KD - 1),
                    )
            qx = epool.tile([P, NL * P], f32, tag="qx")
            nc.scalar.activation(
                out=qx,
                in_=ps,
                func=Act.Identity,
                bias=bq_sb[:, m : m + 1],
                scale=1.0,
            )
            elu1(qT[m], qx, epool)
        psq.release()
        epool.release()
        wq_pool.release()
        if not use_cc:
            xnT_pool.release()
        xqnT_pool.release()

        # =========== phase KASM: assemble global kT from the gather ======
        if use_cc:
            for k in range(KD):
                dst = kT[k].rearrange("p (i two c) -> p i two c", i=NL, two=2, c=P)
                for pj in range(2):
                    src = g_kT[pj, k].rearrange("p (i c) -> p i c", i=NL, c=P)
                    nc.sync.dma_start(out=dst[:, :, pj], in_=src)

        def v_chunk(j):
            return g_v[j & 1, j >> 1] if use_cc else vdram[j]

        # =========== phase ATTN: attnT[s,t] = kT.T @ qT, masked ==========
        # s-chunk j is needed by local t-chunks i >= floor(j/2); the first
        # 128 t-cols of each eviction get the parity mask, the rest copy.
        attn_pool = tc.alloc_tile_pool(name="attnT", bufs=1)
        attnT = []
        tstart = []
        for j in range(NT):
            t0 = (j // 2) * P
            tstart.append(t0)
            attnT.append(
                attn_pool.tile(
                    [P, NL * P - t0], f16, tag=f"attnT{j}", name=f"attnT{j}"
                )
            )
        psa = tc.alloc_tile_pool(name="psA", bufs=3, space="PSUM")
        for j in range(NT):
            ntj = NL * P - tstart[j]
            ps = psa.tile([P, 1024], f32, tag="psA")
            for k in range(KD):
                for sub in range(0, ntj, 512):
                    w = min(512, ntj - sub)
                    nc.tensor.matmul(
                        out=ps[:, sub : sub + w],
                        lhsT=kT[k][:, j * P : (j + 1) * P],
                        rhs=qT[k][:, tstart[j] + sub : tstart[j] + sub + w],
                        start=(k == 0),
                        stop=(k == KD - 1),
                    )
            # masked eviction: first 128 cols get the parity mask, rest copy
            rel = j & 1
            nc.vector.tensor_mul(
                attnT[j][:, 0:P], ps[:, 0:P], mask_sb[:, rel * P : (rel + 1) * P]
            )
            if ntj > P:
                nc.scalar.copy(out=attnT[j][:, P:ntj], in_=ps[:, P:ntj])
        psa.release()
        qT_pool.release()
        kT_pool.release()

        # =========== phase OUT: out = (attnT.T @ v_aug), then /den =======
        oacc_pool = tc.alloc_tile_pool(name="oacc", bufs=1)
        out_acc = [
            oacc_pool.tile([P, D + 2], f32, tag=f"oacc{i}", name=f"oacc{i}")
            for i in range(NL)
        ]
        vg_pool = tc.alloc_tile_pool(name="vg", bufs=8)
        fpool = tc.alloc_tile_pool(name="fin", bufs=4)
        pso = tc.alloc_tile_pool(name="psO", bufs=2, space="PSUM")
        for g in range(4):
            vgt = []
            for jj in range(4):
                t = vg_pool.tile([P, D + 2], f16, tag="vg", name="vg")
                nc.sync.dma_start(out=t, in_=v_chunk(4 * g + jj))
                vgt.append(t)
            for i in range(2 * g, NL):
                js = [j for j in range(4 * g, min(4 * g + 4, 2 * i + 2))]
                ps = pso.tile([P, D + 2], f32, tag="psO")
                for idx, j in enumerate(js):
                    acol = (i - j // 2) * P
                    lhs = attnT[j][:, acol : acol + P]
                    for s0, s1 in ((0, 512), (512, 1024), (1024, 1026)):
                        nc.tensor.matmul(
                            out=ps[:, s0:s1],
                            lhsT=lhs,
                            rhs=vgt[j % 4][:, s0:s1],
                            start=(idx == 0),
                            stop=(idx == len(js) - 1),
                        )
                if g == 0:
                    nc.scalar.copy(out=out_acc[i], in_=ps)
                else:
                    nc.vector.tensor_add(out_acc[i], out_acc[i], ps)
                if g == (2 * i + 1) // 4:
                    # finalize row-chunk i: out = num / (den + eps)
                    di = fpool.tile([P, 1], f32, tag="di")
                    nc.vector.tensor_scalar(
                        out=di,
                        in0=out_acc[i][:, D : D + 1],
                        scalar1=DEN_EPS,
                        scalar2=None,
                        op0=Alu.add,
                    )
                    dr = fpool.tile([P, 1], f32, tag="dr")
                    nc.vector.reciprocal(out=dr, in_=di)
                    ofin = fpool.tile([P, D], f16, tag="ofin", name="ofin")
                    nc.vector.tensor_scalar_mul(
                        out=ofin, in0=out_acc[i][:, 0:D], scalar1=dr
                    )
                    nc.sync.dma_start(
                        out=out_d[i * P : (i + 1) * P, :], in_=ofin
                    )
        pso.release()
        fpool.release()
        vg_pool.release()
        oacc_pool.release()
        attn_pool.release()
        const.release()

    return nc


def _prep_weights(inputs):
    qkv_w = np.asarray(inputs["qkv_w"], dtype=np.float32)
    qkv_b = np.asarray(inputs["qkv_b"], dtype=np.float32)
    gate_w = np.asarray(inputs["gate_w"], dtype=np.float32)
    gate_b = np.asarray(inputs["gate_b"], dtype=np.float32)
    ln_g = np.asarray(inputs["ln_g"], dtype=np.float32)
    ln_b = np.asarray(inputs["ln_b"], dtype=np.float32)

    w_eff = qkv_w * ln_g[:, None]
    b_eff = (qkv_b + ln_b @ qkv_w).astype(np.float32)
    wg_eff = gate_w * ln_g[:, None]
    bg_eff = (gate_b + ln_b @ gate_w).astype(np.float32)

    # w[din, dout] -> tiles[m, k] = w[k*P:(k+1)*P, m*P:(m+1)*P]
    def tiles_mk(w):
        return np.ascontiguousarray(w.reshape(KD, P, KD, P).transpose(2, 0, 1, 3))

    wq = tiles_mk(w_eff[:, 0:D])
    wk = tiles_mk(w_eff[:, D : 2 * D])
    wg = tiles_mk(wg_eff)
    wv = np.ascontiguousarray(w_eff[:, 2 * D : 3 * D].reshape(KD, P, D))
    return (
        wq, wk, wg, wv,
        b_eff[0:D], b_eff[D : 2 * D], bg_eff, b_eff[2 * D : 3 * D],
    )


def _build_wsh(wq, wk, wg, wv):
    """Full SBUF-layout weight buffer [P, 4*KD*KD*P] fp16, split into
    N_CORES contiguous column shards (order [wq | wk | wg | wv])."""
    bfn = np.float16

    def sb_order(wt):
        return wt.transpose(2, 0, 1, 3).reshape(P, KD * KD * P)

    full = np.concatenate(
        [
            sb_order(wq),
            sb_order(wk),
            sb_order(wg),
            wv.transpose(1, 0, 2).reshape(P, KD * D),
        ],
        axis=1,
    ).astype(bfn)
    wsh = full.shape[1] // N_CORES
    return [
        np.ascontiguousarray(full[:, c * wsh : (c + 1) * wsh])
        for c in range(N_CORES)
    ]


def _build_masks():
    ss = np.arange(P)
    out = []
    for par in (0, 1):
        m = np.zeros((2, P, P), dtype=np.float32)
        for rel in range(2):
            m[rel] = (ss[:, None] <= ss[None, :] + (par - rel) * P).astype(
                np.float32
            )
        out.append(m)
    return out


def _fingerprint(inputs):
    parts = []
    for k in ("qkv_w", "qkv_b", "gate_w", "gate_b", "ln_g", "ln_b"):
        a = np.asarray(inputs[k])
        r = a.ravel()
        parts.append(
            (k, a.shape, str(a.dtype), r[:64].tobytes(), r[:: max(1, r.size // 97)].tobytes())
        )
    return hash(tuple(parts))


def _get_program(inputs, use_cc=True):
    fp = (_fingerprint(inputs), use_cc)
    if _CACHE.get("fp") != fp:
        w = _prep_weights(inputs)
        _CACHE["nc"] = build_program(*w, use_cc=use_cc)
        _CACHE["fp"] = fp
        _CACHE["masks"] = _build_masks()
        _CACHE["wsh"] = _build_wsh(*w[:4]) if use_cc else None
    return _CACHE["nc"]


def _core_inputs(inputs, with_x=False):
    x = np.asarray(inputs["x"], dtype=np.float32)
    masks = _CACHE["masks"]
    wsh = _CACHE["wsh"]
    core_inputs = []
    for b in range(B):
        xb = x[b].reshape(NT, P, D)
        for par in (0, 1):
            c = 2 * b + par
            m = {"xq": xb[par::2], "masks": masks[par]}
            if wsh is not None:
                m["wsh"] = wsh[c]
            if with_x:
                m["x"] = xb
            core_inputs.append(m)
    return core_inputs


def _host_assemble(results):
    out = np.empty((B, T, D), dtype=np.float32)
    for c in range(N_CORES):
        b, par = c >> 1, c & 1
        out[b].reshape(NT, P, D)[par::2] = results[c]["out"].reshape(NL, P, D)
    return out


def _enable_pcc():
    """Persistent jax compilation cache: warm calls skip backend compile."""
    if _CACHE.get("pcc"):
        return
    import jax

    try:
        jax.config.update("jax_compilation_cache_dir", "/tmp/jax_pcc_cgd2")
        jax.config.update("jax_persistent_cache_min_compile_time_secs", 0)
        jax.config.update("jax_persistent_cache_min_entry_size_bytes", 0)
    except Exception:
        pass
    _CACHE["pcc"] = True


def kernel(**inputs):
    from concourse.bass_utils import run_bass_kernel_spmd

    _enable_pcc()
    nc = _get_program(inputs, use_cc=True)
    core_inputs = _core_inputs(inputs)
    res = run_bass_kernel_spmd(nc, core_inputs, list(range(N_CORES)))
    return _host_assemble(res.results)


# revision 24
# speedup vs baseline: 1.1254x; 1.1254x over previous
"""Trainium2 Bass kernel for CausalGatedD2Attention.

Math (per batch b):
  xn   = LayerNorm(x) * ln_g + ln_b            [T, D]
  qkv  = xn @ qkv_w + qkv_b                     -> q, k, v  [T, D] each
  gate = sigmoid(xn @ gate_w + gate_b)
  k    = elu(k * gate) + 1 ;  q = elu(q) + 1
  attn = tril(q @ k^T)                          [T, T]
  out  = (attn @ v) / (rowsum(attn) + eps)      [T, D]
(rowsum(attn) == sum(q * cumsum(k), -1) under the causal mask.)

Sharding: 4 batches x 2 cores.  Within a pair, core parity par in {0,1}
owns the even/odd 128-row t-chunks of its batch (balances the causal
triangle).  Each core LayerNorms + projects ONLY its own 1024 rows;
k (gated, elu+1) and v (with an appended [1,0] denominator column) for
the peer's rows arrive via a pair-wise DRAM AllGather.  All 8 cores run
ONE uniform program; causality is applied via two host-provided
[128,128] mask tiles (content depends only on core parity), so the
instruction stream is identical across cores - only input data differs.

Weights are folded with ln_g/ln_b on the host, cast to fp16, and
shipped as a per-core 1 MB column shard of the SBUF-layout weight
buffer; an 8-core AllGather reassembles the full 8 MB on every device
at kernel start (overlapped with the LayerNorm phase).  Biases ride as
tiny runtime inputs, so the BIR -- and with it the NEFF cache key and
the jax persistent-compilation-cache key -- is fully independent of the
input values; recompiles only happen if this file changes.  All matmul
operands live in
fp16 (1 cycle/row on the PE, same as bf16, with 8 more mantissa bits);
LayerNorm statistics, activations and the num/den accumulators stay
f32.  Per-call wire traffic is xq (the core's own 1024 rows, f32,
passed as a strided view of x), the 1 MB weight shard, a 128 KB mask
pair in, and the 2 MB fp16 output slice back -- ~57 MB total vs
~296 MB for the naive replication (the host-side assembly casts fp16 ->
f32 during the strided scatter, so the down-cast is free).  The jax
persistent compilation cache is enabled so warm calls skip the backend
compile (bir verify + neuronx-cc) entirely.

The denominator comes for free: v gets an appended ones-column, so
attn @ v_aug yields [num | den] in one accumulation.
"""

import sys

sys.path.insert(0, "/opt/trn_rl_repo")

import numpy as np

B, T, D = 4, 2048, 1024
P = 128
KD = D // P          # 8 contraction chunks
NT = T // P          # 16 global t-chunks
NL = NT // 2         # 8 local t-chunks per core
LN_EPS = 1e-5
DEN_EPS = 1e-6
N_CORES = 8

_CACHE = {}


def _patched_tc(tile_mod):
    import bass_rust as _br
    from concourse.vector_clock import ScopedClock

    class TC(tile_mod.TileContext):
        """TileContext whose final drain splits sem waits one per
        instruction (walrus CoreV3 allows a single wait on Drain)."""

        def _spread_waits(self):
            # walrus allows at most 2 sem waits on engine instructions and
            # only 1 on CTRL-class ones (Drain/NoOp); Tile's scheduler can
            # emit more.  Move excess waits onto same-engine nops placed
            # immediately before the over-limit instruction.
            nc = self.nc
            for fnbb in nc.m.functions[0].blocks:
                insts = list(fnbb.instructions)
                out = []
                for inst in insts:
                    si = inst.sync_info
                    waits = list(si.on_wait) if si is not None else []
                    limit = 1
                    if len(waits) > limit:
                        excess = waits[limit:]
                        si.on_wait = waits[:limit]
                        inst.sync_info = si
                        for w in excess:
                            nop = nc.engines[inst.engine].nop(
                                nofuse=True, hint="wait_spread"
                            )
                            nop.ins.sync_info = _br.SyncInfo(
                                on_wait=[w], on_update=[]
                            )
                            # remove from wherever it was appended
                            for b2 in nc.m.functions[0].blocks:
                                cur = list(b2.instructions)
                                if cur and cur[-1] is nop.ins:
                                    b2.instructions = cur[:-1]
                                    break
                            out.append(nop.ins)
                    out.append(inst)
                fnbb.instructions = out

        def _drain_and_barrier(self, tick_clock, wait_clock):
            self._spread_waits()
            drain_inst = self.nc.sync.drain()
            wait_clock.add_sem_waits(
                drain_inst.ins, ScopedClock({None: tick_clock.global_clock})
            )
            si = drain_inst.ins.sync_info
            waits = list(si.on_wait)
            if len(waits) > 1:
                si.on_wait = waits[:1]
                drain_inst.ins.sync_info = si
                for i in range(1, len(waits)):
                    nop = self.nc.sync.nop(nofuse=True, hint="drain_extra_waits")
                    nop.ins.sync_info = _br.SyncInfo(
                        on_wait=waits[i : i + 1], on_update=[]
                    )
            self.nc.all_engine_barrier()
            assert self.sems is not None
            popped = self.nc._tile_sem_poison_stack.pop()
            assert popped is self._sem_poison
            self.nc.clear_and_free_semaphores(list(self.sems.allocated().values()))
            self.nc.all_engine_barrier()

    return TC


def build_program(wq, wk, wg, wv, bq, bk, bg, bv, use_cc=True):
    """wq/wk/wg: [KD,KD,P,P] f32 tiles (m,k);  wv: [KD,P,D] f32;
    bq/bk/bg: [D] f32;  bv: [D] f32."""
    import concourse.bass as bass
    import concourse.tile as tile
    from concourse import mybir
    from concourse.masks import make_identity

    TC = _patched_tc(tile)
    f32 = mybir.dt.float32
    f16 = mybir.dt.float16
    Act = mybir.ActivationFunctionType
    Alu = mybir.AluOpType
    bfn = np.float16

    nc = bass.Bass(num_devices=N_CORES)
    xq_in = nc.declare_dram_parameter("xq", [NL, P, D], f32, isOutput=False)
    masks_in = nc.declare_dram_parameter("masks", [2, P, P], f32, isOutput=False)
    out_d = nc.declare_dram_parameter("out", [NL * P, D], f16, isOutput=True)
    if not use_cc:
        x_in = nc.declare_dram_parameter("x", [NT, P, D], f32, isOutput=False)

    # weights pre-arranged for single-DMA SBUF residency:
    #   [m,k,p,q] -> [p, (m k q)]   (lhsT tile (m,k) = [:, (m*KD+k)*P : +P])
    def sb_order(wt):
        return np.ascontiguousarray(
            wt.transpose(2, 0, 1, 3).reshape(P, KD * KD * P).astype(bfn)
        )

    WSH = 4 * KD * KD * P // N_CORES  # 4096-col shard of the weight buffer
    if use_cc:
        # weights arrive as a per-core 1 MB fp16 shard and are AllGathered
        # on device; [wq | wk | wg | wv] each spans two 4096-col shards.
        wsh_in = nc.declare_dram_parameter("wsh", [P, WSH], f16, isOutput=False)
        own_w = nc.dram_tensor("own_w", [P, WSH], f16)
        g_w = nc.dram_tensor("g_w", [N_CORES, P, WSH], f16, addr_space="Shared")
    else:
        wq_i = nc.inline_tensor(sb_order(wq), name="wq_i")
        wk_i = nc.inline_tensor(sb_order(wk), name="wk_i")
        wg_i = nc.inline_tensor(sb_order(wg), name="wg_i")
        wv_i = nc.inline_tensor(
            np.ascontiguousarray(
                wv.transpose(1, 0, 2).reshape(P, KD * D).astype(bfn)
            ),
            name="wv_i",
        )
    # biases ride as tiny runtime inputs so the BIR (and thus the NEFF
    # cache key) is independent of the weight values:
    #   b3 [P, 3*KD]: column m of group g = bias_g[m*128:(m+1)*128]
    b3_in = nc.declare_dram_parameter("b3", [P, 3 * KD], f32, isOutput=False)
    bv_in = nc.declare_dram_parameter("bv", [1, D], f32, isOutput=False)

    own_kT = nc.dram_tensor("own_kT", [KD, P, NL * P], f16)
    own_v = nc.dram_tensor("own_v", [NL, P, D + 2], f16)
    if use_cc:
        g_kT = nc.dram_tensor("g_kT", [2, KD, P, NL * P], f16)
        g_v = nc.dram_tensor("g_v", [2, NL, P, D + 2], f16)
        groups = [[2 * i, 2 * i + 1] for i in range(4)]
        groups8 = [list(range(N_CORES))]
    else:
        vdram = nc.dram_tensor("vdram", [NT, P, D + 2], f16)

    with TC(nc) as tc:
        if use_cc:
            # kick off the weight AllGather first; it overlaps with LNQ
            nc.sync.dma_start(out=own_w[:], in_=wsh_in[:])
            nc.gpsimd.collective_compute(
                "AllGather",
                mybir.AluOpType.bypass,
                replica_groups=groups8,
                ins=[own_w[:].opt()],
                outs=[g_w[:].opt()],
            )

        def load_w_sb(dst, widx):
            """dst [P, 8192] <- weight widx (0=q,1=k,2=g,3=v)."""
            if use_cc:
                for h in range(2):
                    nc.sync.dma_start(
                        out=dst[:, h * WSH : (h + 1) * WSH],
                        in_=g_w[2 * widx + h],
                    )
            else:
                nc.sync.dma_start(
                    out=dst, in_=(wq_i, wk_i, wg_i, wv_i)[widx][:]
                )

        const = tc.alloc_tile_pool(name="const", bufs=1)
        ident = const.tile([P, P], f16, tag="ident")
        make_identity(nc, ident)
        mask_sb = const.tile([P, 2 * P], f32, tag="mask")
        for rel in range(2):
            nc.sync.dma_start(
                out=mask_sb[:, rel * P : (rel + 1) * P], in_=masks_in[rel]
            )
        b3_sb = const.tile([P, 3 * KD], f32, tag="b3")
        nc.sync.dma_start(out=b3_sb, in_=b3_in[:])
        bq_sb = b3_sb[:, 0:KD]
        bk_sb = b3_sb[:, KD : 2 * KD]
        bg_sb = b3_sb[:, 2 * KD : 3 * KD]
        vb_sb = const.tile([P, D], f32, tag="vb")
        bvs = bv_in[0]
        vb_bcast = bass.AP(tensor=bvs.tensor, offset=bvs.offset, ap=[[0, P], *bvs.ap])
        nc.sync.dma_start(out=vb_sb, in_=vb_bcast)
        ln_eps = const.tile([P, 1], f32, tag="lneps")
        nc.vector.memset(ln_eps, LN_EPS)
        onez_sb = const.tile([P, 2], f16, tag="onez")
        nc.vector.memset(onez_sb[:, 0:1], 1.0)
        nc.vector.memset(onez_sb[:, 1:2], 0.0)

        # ---- helper: layernorm one 128-row chunk + transpose into dstT ----
        def ln_transpose(src, c, dstT, xpool, spool, pspool):
            xt = xpool.tile([P, D], f32, tag="xt")
            nc.sync.dma_start(out=xt, in_=src)
            stats = spool.tile([P, 2, 6], f32, tag="stats")
            xr = xt.rearrange("p (n f) -> p n f", n=2)
            for sg in range(2):
                nc.vector.bn_stats(out=stats[:, sg], in_=xr[:, sg])
            mv = spool.tile([P, 2], f32, tag="mv")
            nc.vector.bn_aggr(out=mv, in_=stats)
            rstd = spool.tile([P, 1], f32, tag="rstd")
            nc.scalar.activation(
                out=rstd, in_=mv[:, 1:2], func=Act.Sqrt, bias=ln_eps, scale=1.0
            )
            rstd2 = spool.tile([P, 1], f32, tag="rstd2")
            nc.vector.reciprocal(out=rstd2, in_=rstd)
            nmr = spool.tile([P, 1], f32, tag="nmr")
            nc.vector.tensor_scalar(
                out=nmr,
                in0=mv[:, 0:1],
                scalar1=rstd2,
                scalar2=-1.0,
                op0=Alu.mult,
                op1=Alu.mult,
            )
            xn = xpool.tile([P, D], f16, tag="xn")
            nc.scalar.activation(
                out=xn, in_=xt, func=Act.Identity, bias=nmr, scale=rstd2
            )
            for k in range(KD):
                ps = pspool.tile([P, P], f16, tag="psT")
                nc.tensor.transpose(
                    out=ps, in_=xn[:, k * P : (k + 1) * P], identity=ident
                )
                if k % 2 == 0:
                    nc.vector.tensor_copy(dstT[k][:, c * P : (c + 1) * P], ps)
                else:
                    nc.scalar.copy(out=dstT[k][:, c * P : (c + 1) * P], in_=ps)

        # ---- helper: x_elu = elu(x)+1 = max(x,0) + exp(min(x,0)) ----
        def elu1(dst, src, epool):
            m0 = epool.tile(list(src.shape), f32, tag="m0")
            nc.gpsimd.tensor_scalar_min(out=m0, in0=src, scalar1=0.0)
            e = epool.tile(list(src.shape), f32, tag="e")
            nc.scalar.activation(out=e, in_=m0, func=Act.Exp)
            nc.vector.scalar_tensor_tensor(
                out=dst, in0=src, scalar=0.0, in1=e, op0=Alu.max, op1=Alu.add
            )

        # =========== phase LNQ: layernorm + transpose xq -> xqnT =========
        xqnT_pool = tc.alloc_tile_pool(name="xqnT", bufs=1)
        xqnT = [
            xqnT_pool.tile([P, NL * P], f16, tag=f"xqnT{k}", name=f"xqnT{k}")
            for k in range(KD)
        ]
        if not use_cc:
            xnT_pool = tc.alloc_tile_pool(name="xnT", bufs=1)
            xnT = [
                xnT_pool.tile([P, T], f16, tag=f"xnT{k}", name=f"xnT{k}")
                for k in range(KD)
            ]
        xpool = tc.alloc_tile_pool(name="qwork", bufs=3)
        spool = tc.alloc_tile_pool(name="qstat", bufs=4)
        pspool = tc.alloc_tile_pool(name="psTq", bufs=4, space="PSUM")
        for c in range(NL):
            ln_transpose(xq_in[c], c, xqnT, xpool, spool, pspool)
        if not use_cc:
            for c in range(NT):
                ln_transpose(x_in[c], c, xnT, xpool, spool, pspool)
        pspool.release()
        spool.release()
        xpool.release()

        kg_src = xqnT if use_cc else xnT
        kg_cols = NL * P if use_cc else T
        kg_sc = kg_cols // 512

        # kT holds the FULL 2048 key columns in global chunk order
        kT_pool = tc.alloc_tile_pool(name="kT", bufs=1, side="right")
        kT = [
            kT_pool.tile([P, T], f16, tag=f"kT{m}", name=f"kT{m}")
            for m in range(KD)
        ]

        # =========== phase KG: k/gate projections (own rows) =============
        wkg_pool = tc.alloc_tile_pool(name="wkg", bufs=1)
        wk_sb = wkg_pool.tile([P, KD * KD * P], f16, tag="wk_sb", name="wk_sb")
        wg_sb = wkg_pool.tile([P, KD * KD * P], f16, tag="wg_sb", name="wg_sb")
        load_w_sb(wk_sb, 1)
        load_w_sb(wg_sb, 2)
        epool = tc.alloc_tile_pool(name="kgev", bufs=2)
        kopool = tc.alloc_tile_pool(name="kout", bufs=2)
        pskg = tc.alloc_tile_pool(name="psKG", bufs=1, space="PSUM")
        for m in range(KD):
            psK = pskg.tile([P, kg_sc, 512], f32, tag="psK")
            psG = pskg.tile([P, kg_sc, 512], f32, tag="psG")
            for k in range(KD):
                wkt = wk_sb[:, (m * KD + k) * P : (m * KD + k + 1) * P]
                wgt = wg_sb[:, (m * KD + k) * P : (m * KD + k + 1) * P]
                for sc in range(kg_sc):
                    nc.tensor.matmul(
                        out=psK[:, sc],
                        lhsT=wkt,
                        rhs=kg_src[k][:, sc * 512 : (sc + 1) * 512],
                        start=(k == 0),
                        stop=(k == KD - 1),
                    )
                    nc.tensor.matmul(
                        out=psG[:, sc],
                        lhsT=wgt,
                        rhs=kg_src[k][:, sc * 512 : (sc + 1) * 512],
                        start=(k == 0),
                        stop=(k == KD - 1),
                    )
            if use_cc:
                kto = kopool.tile([P, kg_cols], f16, tag="kto", name="kto")
            else:
                kto = kT[m]
            psKf = psK.rearrange("p s c -> p (s c)")
            psGf = psG.rearrange("p s c -> p (s c)")
            for sc in range(kg_sc // 2):
                cols = slice(sc * 1024, (sc + 1) * 1024)
                g = epool.tile([P, 1024], f32, tag="g")
                nc.scalar.activation(
                    out=g,
                    in_=psGf[:, cols],
                    func=Act.Sigmoid,
                    bias=bg_sb[:, m : m + 1],
                    scale=1.0,
                )
                kg = epool.tile([P, 1024], f32, tag="kg")
                nc.vector.scalar_tensor_tensor(
                    out=kg,
                    in0=psKf[:, cols],
                    scalar=bk_sb[:, m : m + 1],
                    in1=g,
                    op0=Alu.add,
                    op1=Alu.mult,
                )
                elu1(kto[:, cols], kg, epool)
            if use_cc:
                nc.sync.dma_start(out=own_kT[m], in_=kto)
        pskg.release()
        kopool.release()
        epool.release()
        wkg_pool.release()
        if use_cc:
            nc.gpsimd.collective_compute(
                "AllGather",
                mybir.AluOpType.bypass,
                replica_groups=groups,
                ins=[own_kT[:].opt()],
                outs=[g_kT[:].opt()],
            )

        # =========== phase V: v projection (own rows) ====================
        wv_pool = tc.alloc_tile_pool(name="wv", bufs=1)
        wv_sb = wv_pool.tile([P, KD * D], f16, tag="wv_sb", name="wv_sb")
        load_w_sb(wv_sb, 3)
        vpool = tc.alloc_tile_pool(name="vev", bufs=3)
        psv = tc.alloc_tile_pool(name="psV", bufs=3, space="PSUM")
        v_src = xqnT if use_cc else xnT
        v_chunks = NL if use_cc else NT
        for s in range(v_chunks):
            ps = psv.tile([P, D], f32, tag="psV")
            for k in range(KD):
                for dc in range(2):
                    nc.tensor.matmul(
                        out=ps[:, dc * 512 : (dc + 1) * 512],
                        lhsT=v_src[k][:, s * P : (s + 1) * P],
                        rhs=wv_sb[:, k * D + dc * 512 : k * D + (dc + 1) * 512],
                        start=(k == 0),
                        stop=(k == KD - 1),
                    )
            vsb = vpool.tile([P, D + 2], f16, tag="vsb")
            nc.vector.tensor_add(vsb[:, 0:D], ps, vb_sb)
            nc.vector.tensor_copy(vsb[:, D : D + 2], onez_sb)
            nc.sync.dma_start(out=own_v[s] if use_cc else vdram[s], in_=vsb)
        psv.release()
        vpool.release()
        wv_pool.release()
        if use_cc:
            nc.gpsimd.collective_compute(
                "AllGather",
                mybir.AluOpType.bypass,
                replica_groups=groups,
                ins=[own_v[:].opt()],
                outs=[g_v[:].opt()],
            )

        # =========== phase QP: q projection -> qT (elu+1) ================
        qT_pool = tc.alloc_tile_pool(name="qT", bufs=1, side="right")
        qT = [
            qT_pool.tile([P, NL * P], f16, tag=f"qT{m}", name=f"qT{m}")
            for m in range(KD)
        ]
        wq_pool = tc.alloc_tile_pool(name="wq", bufs=1)
        wq_sb = wq_pool.tile([P, KD * KD * P], f16, tag="wq_sb", name="wq_sb")
        load_w_sb(wq_sb, 0)
        epool = tc.alloc_tile_pool(name="qev", bufs=3)
        psq = tc.alloc_tile_pool(name="psQ", bufs=3, space="PSUM")
        for m in range(KD):
            ps = psq.tile([P, NL * P], f32, tag="psQ")
            for k in range(KD):
                wqt = wq_sb[:, (m * KD + k) * P : (m * KD + k + 1) * P]
                for sc in range(2):
                    nc.tensor.matmul(
                        out=ps[:, sc * 512 : (sc + 1) * 512],
                        lhsT=wqt,
                        rhs=xqnT[k][:, sc * 512 : (sc + 1) * 512],
                        start=(k == 0),
                        stop=(k == KD - 1),
                    )
            qx = epool.tile([P, NL * P], f32, tag="qx")
            nc.scalar.activation(
                out=qx,
                in_=ps,
                func=Act.Identity,
                bias=bq_sb[:, m : m + 1],
                scale=1.0,
            )
            elu1(qT[m], qx, epool)
        psq.release()
        epool.release()
        wq_pool.release()
        if not use_cc:
            xnT_pool.release()
        xqnT_pool.release()

        # =========== phase KASM: assemble global kT from the gather ======
        if use_cc:
            for k in range(KD):
                dst = kT[k].rearrange("p (i two c) -> p i two c", i=NL, two=2, c=P)
                for pj in range(2):
                    src = g_kT[pj, k].rearrange("p (i c) -> p i c", i=NL, c=P)
                    nc.sync.dma_start(out=dst[:, :, pj], in_=src)

        def v_chunk(j):
            return g_v[j & 1, j >> 1] if use_cc else vdram[j]

        # =========== phase ATTN: attnT[s,t] = kT.T @ qT, masked ==========
        # s-chunk j is needed by local t-chunks i >= floor(j/2); the first
        # 128 t-cols of each eviction get the parity mask, the rest copy.
        attn_pool = tc.alloc_tile_pool(name="attnT", bufs=1)
        attnT = []
        tstart = []
        for j in range(NT):
            t0 = (j // 2) * P
            tstart.append(t0)
            attnT.append(
                attn_pool.tile(
                    [P, NL * P - t0], f16, tag=f"attnT{j}", name=f"attnT{j}"
                )
            )
        psa = tc.alloc_tile_pool(name="psA", bufs=3, space="PSUM")
        for j in range(NT):
            ntj = NL * P - tstart[j]
            ps = psa.tile([P, 1024], f32, tag="psA")
            for k in range(KD):
                for sub in range(0, ntj, 512):
                    w = min(512, ntj - sub)
                    nc.tensor.matmul(
                        out=ps[:, sub : sub + w],
                        lhsT=kT[k][:, j * P : (j + 1) * P],
                        rhs=qT[k][:, tstart[j] + sub : tstart[j] + sub + w],
                        start=(k == 0),
                        stop=(k == KD - 1),
                    )
            # masked eviction: first 128 cols get the parity mask, rest copy
            rel = j & 1
            nc.vector.tensor_mul(
                attnT[j][:, 0:P], ps[:, 0:P], mask_sb[:, rel * P : (rel + 1) * P]
            )
            if ntj > P:
                nc.scalar.copy(out=attnT[j][:, P:ntj], in_=ps[:, P:ntj])
        psa.release()
        qT_pool.release()
        kT_pool.release()

        # =========== phase OUT: out = (attnT.T @ v_aug), then /den =======
        oacc_pool = tc.alloc_tile_pool(name="oacc", bufs=1)
        out_acc = [
            oacc_pool.tile([P, D + 2], f32, tag=f"oacc{i}", name=f"oacc{i}")
            for i in range(NL)
        ]
        vg_pool = tc.alloc_tile_pool(name="vg", bufs=8)
        fpool = tc.alloc_tile_pool(name="fin", bufs=4)
        pso = tc.alloc_tile_pool(name="psO", bufs=2, space="PSUM")
        for g in range(4):
            vgt = []
            for jj in range(4):
                t = vg_pool.tile([P, D + 2], f16, tag="vg", name="vg")
                nc.sync.dma_start(out=t, in_=v_chunk(4 * g + jj))
                vgt.append(t)
            for i in range(2 * g, NL):
                js = [j for j in range(4 * g, min(4 * g + 4, 2 * i + 2))]
                ps = pso.tile([P, D + 2], f32, tag="psO")
                for idx, j in enumerate(js):
                    acol = (i - j // 2) * P
                    lhs = attnT[j][:, acol : acol + P]
                    for s0, s1 in ((0, 512), (512, 1024), (1024, 1026)):
                        nc.tensor.matmul(
                            out=ps[:, s0:s1],
                            lhsT=lhs,
                            rhs=vgt[j % 4][:, s0:s1],
                            start=(idx == 0),
                            stop=(idx == len(js) - 1),
                        )
                if g == 0:
                    nc.scalar.copy(out=out_acc[i], in_=ps)
                else:
                    nc.vector.tensor_add(out_acc[i], out_acc[i], ps)
                if g == (2 * i + 1) // 4:
                    # finalize row-chunk i: out = num / (den + eps)
                    di = fpool.tile([P, 1], f32, tag="di")
                    nc.vector.tensor_scalar(
                        out=di,
                        in0=out_acc[i][:, D : D + 1],
                        scalar1=DEN_EPS,
                        scalar2=None,
                        op0=Alu.add,
                    )
                    dr = fpool.tile([P, 1], f32, tag="dr")
                    nc.vector.reciprocal(out=dr, in_=di)
                    ofin = fpool.tile([P, D], f16, tag="ofin", name="ofin")
                    nc.vector.tensor_scalar_mul(
                        out=ofin, in0=out_acc[i][:, 0:D], scalar1=dr
                    )
                    nc.sync.dma_start(
                        out=out_d[i * P : (i + 1) * P, :], in_=ofin
                    )
        pso.release()
        fpool.release()
        vg_pool.release()
        oacc_pool.release()
        attn_pool.release()
        const.release()

    return nc


def _prep_weights(inputs):
    qkv_w = np.asarray(inputs["qkv_w"], dtype=np.float32)
    qkv_b = np.asarray(inputs["qkv_b"], dtype=np.float32)
    gate_w = np.asarray(inputs["gate_w"], dtype=np.float32)
    gate_b = np.asarray(inputs["gate_b"], dtype=np.float32)
    ln_g = np.asarray(inputs["ln_g"], dtype=np.float32)
    ln_b = np.asarray(inputs["ln_b"], dtype=np.float32)

    w_eff = qkv_w * ln_g[:, None]
    b_eff = (qkv_b + ln_b @ qkv_w).astype(np.float32)
    wg_eff = gate_w * ln_g[:, None]
    bg_eff = (gate_b + ln_b @ gate_w).astype(np.float32)

    # w[din, dout] -> tiles[m, k] = w[k*P:(k+1)*P, m*P:(m+1)*P]
    def tiles_mk(w):
        return np.ascontiguousarray(w.reshape(KD, P, KD, P).transpose(2, 0, 1, 3))

    wq = tiles_mk(w_eff[:, 0:D])
    wk = tiles_mk(w_eff[:, D : 2 * D])
    wg = tiles_mk(wg_eff)
    wv = np.ascontiguousarray(w_eff[:, 2 * D : 3 * D].reshape(KD, P, D))
    return (
        wq, wk, wg, wv,
        b_eff[0:D], b_eff[D : 2 * D], bg_eff, b_eff[2 * D : 3 * D],
    )


def _build_wsh(wq, wk, wg, wv):
    """Full SBUF-layout weight buffer [P, 4*KD*KD*P] fp16, split into
    N_CORES contiguous column shards (order [wq | wk | wg | wv])."""
    bfn = np.float16

    def sb_order(wt):
        return wt.transpose(2, 0, 1, 3).reshape(P, KD * KD * P)

    full = np.concatenate(
        [
            sb_order(wq),
            sb_order(wk),
            sb_order(wg),
            wv.transpose(1, 0, 2).reshape(P, KD * D),
        ],
        axis=1,
    ).astype(bfn)
    wsh = full.shape[1] // N_CORES
    return [
        np.ascontiguousarray(full[:, c * wsh : (c + 1) * wsh])
        for c in range(N_CORES)
    ]


def _build_masks():
    ss = np.arange(P)
    out = []
    for par in (0, 1):
        m = np.zeros((2, P, P), dtype=np.float32)
        for rel in range(2):
            m[rel] = (ss[:, None] <= ss[None, :] + (par - rel) * P).astype(
                np.float32
            )
        out.append(m)
    return out


def _fingerprint(inputs):
    parts = []
    for k in ("qkv_w", "qkv_b", "gate_w", "gate_b", "ln_g", "ln_b"):
        a = np.asarray(inputs[k])
        r = a.ravel()
        parts.append(
            (k, a.shape, str(a.dtype), r[:64].tobytes(), r[:: max(1, r.size // 97)].tobytes())
        )
    return hash(tuple(parts))


def _get_program(inputs, use_cc=True):
    fp = (_fingerprint(inputs), use_cc)
    if _CACHE.get("fp") != fp:
        w = _prep_weights(inputs)
        _CACHE["nc"] = build_program(*w, use_cc=use_cc)
        _CACHE["fp"] = fp
        _CACHE["masks"] = _build_masks()
        _CACHE["wsh"] = _build_wsh(*w[:4]) if use_cc else None
        bq, bk, bg, bv = w[4:]
        _CACHE["b3"] = np.ascontiguousarray(
            np.concatenate(
                [b.reshape(KD, P).T for b in (bq, bk, bg)], axis=1
            ).astype(np.float32)
        )
        _CACHE["bv"] = np.ascontiguousarray(bv.reshape(1, D).astype(np.float32))
    return _CACHE["nc"]


def _core_inputs(inputs, with_x=False):
    x = np.asarray(inputs["x"], dtype=np.float32)
    masks = _CACHE["masks"]
    wsh = _CACHE["wsh"]
    core_inputs = []
    for b in range(B):
        xb = x[b].reshape(NT, P, D)
        for par in (0, 1):
            c = 2 * b + par
            m = {
                "xq": xb[par::2],
                "masks": masks[par],
                "b3": _CACHE["b3"],
                "bv": _CACHE["bv"],
            }
            if wsh is not None:
                m["wsh"] = wsh[c]
            if with_x:
                m["x"] = xb
            core_inputs.append(m)
    return core_inputs


def _host_assemble(results):
    out = np.empty((B, T, D), dtype=np.float32)
    for c in range(N_CORES):
        b, par = c >> 1, c & 1
        out[b].reshape(NT, P, D)[par::2] = results[c]["out"].reshape(NL, P, D)
    return out


def _enable_pcc():
    """Persistent jax compilation cache: warm calls skip backend compile."""
    if _CACHE.get("pcc"):
        return
    import jax

    try:
        jax.config.update("jax_compilation_cache_dir", "/tmp/jax_pcc_cgd2")
        jax.config.update("jax_persistent_cache_min_compile_time_secs", 0)
        jax.config.update("jax_persistent_cache_min_entry_size_bytes", 0)
    except Exception:
        pass
    _CACHE["pcc"] = True


def kernel(**inputs):
    from concourse.bass_utils import run_bass_kernel_spmd

    _enable_pcc()
    nc = _get_program(inputs, use_cc=True)
    core_inputs = _core_inputs(inputs)
    res = run_bass_kernel_spmd(nc, core_inputs, list(range(N_CORES)))
    return _host_assemble(res.results)
